# revision 38
# baseline (speedup 1.0000x reference)
"""BlockAttentionResidual Trainium2 kernel (custom-DVE 2x edition).

Math (per token t, feature dim D=1024, over N+1=9 blocks):
    ssq[n,t]  = sum_d v[n,t,d]^2
    rq[n,t]   = (ssq/D + eps)^(-1/2)        (computed as exp(-0.5*ln(ssq/D+eps)))
    logit     = (sum_d w2[d]*v[n,t,d]) * rq      where w2 = proj_w*norm_w
    w[n,t]    = softmax over n of logit
    h[t,d]    = sum_n w[n,t] * v[n,t,d]

Sharding: B*T = 8192 tokens split evenly across 8 cores (1024 tokens/core).
Host casts bulk data to bf16 and pre-interleaves the 9 blocks into
vstack[oct, p, (g,d)] with partition p = 14*n + t' (126 rows) and 8
token-groups of 14 in the free dim (one oct = 112 tokens).

The stat reductions (dot = sum v*w2, ssq = sum v^2) are free-dim
reductions, which the stock accumulating DVE/ACT ops only run at
1 elem/cycle (no perf-mode uops for the accumulating variants, plus an
~80/286ns READ_ACCUMULATOR drain per instruction).  This kernel instead
uses a hand-written custom DVE op (per-NEFF uop table) that computes the
reduction as a running PREFIX into the out stream at the 2X_1PORT rate
(two packed bf16 per port per cycle):

    PFXDOT_ANT: out[k] = sum_{j<=k} in0[j]*in1[j]   (fp16 out)

The group total is then just out[:, -1] — no accumulator state and no
READ_ACCUMULATOR (whose handler doesn't reset the perf-mode register and
mangles its store under 2x — HW-verified).  Per FD=1024 group: ~690ns vs
~1300ns (stock stt+read) on DVE / ~1420ns (Square+accum+read) on ACT.
ssq on DVE is the same op with in1=in0.  fp16 totals keep the stat
rounding at ~5e-4 relative, negligible vs the bf16 data path.

Engine split per oct (knobs below): 8 dot groups + (8-K_ACT) ssq groups
on DVE via PFXDOT; K_ACT ssq groups on ACT via Square+accum(f32); PSUM->
SBUF h copies split ACT/DVE; softmax smalls on ACT/DVE; weighted h sum
on PE (mask-matmul trick) as before.
"""

import os
import sys
from operator import add

import numpy as np

for _p in ("/opt/trn_rl_repo", "/root/.axon_site/_ro/trn_rl_repo"):
    if os.path.isdir(_p) and _p not in sys.path:
        sys.path.append(_p)

N_CORES = 8
N, B, T, D = 8, 4, 2048, 1024
EPS = 1e-6
TOK = (B * T) // N_CORES          # 1024 tokens per core
NB = N + 1                        # 9 stacked blocks
GROUP = 14                        # tokens per group (14*9 = 126 <= 128)
ROWS = GROUP * NB                 # 126 used partitions
QG = 8                            # groups per oct (two PSUM pages)
QTOK = GROUP * QG                 # 112 tokens per oct
NQUAD = (TOK + QTOK - 1) // QTOK  # 10 (last oct ragged: 16 real tokens)

# ssq groups computed on ACT (rest on DVE via PFXDOT): engine balance knob
K_ACT = int(os.environ.get("BLOCKATTN_K_ACT", "4"))
# columns of each PSUM h-page copied by ACT (rest by DVE)
ACT_COPY_COLS = int(os.environ.get("BLOCKATTN_ACT_COPY", "1024"))
ACT_SET = "natural_log_exp_and_others"

_CACHE = {}


# --------------------------------------------------------------------------
# custom DVE op: prefix multiply-accumulate at 2X_1PORT
# --------------------------------------------------------------------------

def _register_pfxdot():
    """Define + register PFXDOT_ANT (idempotent). Returns the op."""
    import concourse.dve_ops as dom
    from concourse.dve_spec import (
        Spec, Src0, Src1, AluOp as SAluOp, lower, _has_src1, scan)
    from concourse.dve_uop import (
        AluInp, AluOp, DveOpSpec, ENABLE, InpSel, OutPath, OutSel, Trigger,
        UopConfig)

    if "PFXDOT_ANT" in dom._SUB_OPCODE_FOR_NAME:
        return dom._PFXDOT_ANT_OP

    def _pfx_ref(in0, in1, c0, c1, c2):
        b = (in0.astype(np.float32) * in1.astype(np.float32)).astype(np.float32)
        return np.cumsum(b.reshape(b.shape[0], -1), axis=-1).reshape(b.shape)

    def _seed_zero():
        # one non-consuming cycle: 0.0 -> blk6 out-flop (the recurrence seed)
        u = UopConfig()
        for i, src in ((1, InpSel.SRC_0), (2, InpSel.SRC_1),
                       (3, InpSel.SRC_0_HI), (4, InpSel.SRC_1_HI),
                       (5, InpSel.ZERO)):
            u.enable_input(src, i)
        for k in range(6):
            u.datapath_config[k].pass_through_delay(4)
        u.datapath_config[6].enable_alu(
            AluOp.BYPASS, AluInp.PREV_DELAY_4)
        u.datapath_config[7].enable_alu(AluOp.BYPASS, AluInp.PREV_ALU_OUT)
        u.trigger = (Trigger.COUNT, Trigger.NONE, Trigger.NONE)
        u.repeat_count = 1
        u.next_uop = (1, 0, 0)
        return u

    def _steady_2x():
        # per cycle: acc += x_lo*y_lo + x_hi*y_hi (recurrence at blk6);
        # write the running prefix to both 16-bit halves (W0H via a delay —
        # one OutSel source cannot feed two write paths; HW-verified).
        u = UopConfig()
        for i, src in ((1, InpSel.SRC_0), (2, InpSel.SRC_1),
                       (3, InpSel.SRC_0_HI), (4, InpSel.SRC_1_HI),
                       (5, InpSel.ZERO)):
            u.enable_input(src, i)
        dp = u.datapath_config
        dp[0].enable_alu(AluOp.MULTIPLY, AluInp.PREV_DELAY_0,
                         AluInp.PREV_DELAY_1)
        dp[0].pass_through_delay(2, 3)
        dp[1].enable_alu(AluOp.MULTIPLY, AluInp.PREV_DELAY_2,
                         AluInp.PREV_DELAY_3)
        dp[1].enable_delay_from_src(AluInp.PREV_ALU_OUT, 0)
        dp[2].enable_alu(AluOp.ADD, AluInp.PREV_ALU_OUT, AluInp.PREV_DELAY_0)
        for k in (3, 4, 5):
            dp[k].enable_alu(AluOp.BYPASS, AluInp.PREV_ALU_OUT)
        dp[6].enable_alu(AluOp.ADD, AluInp.CURR_ALU_OUT, AluInp.PREV_ALU_OUT)
        dp[7].enable_alu(AluOp.BYPASS, AluInp.PREV_ALU_OUT)
        dp[7].enable_delay_from_src(AluInp.PREV_ALU_OUT, 0)
        u.enable_output(OutSel.ALU_OUT, OutPath.WR0_LO)
        u.enable_output(OutSel.DELAY_0, OutPath.WR0_HI)
        u.require_inp0 = ENABLE
        u.require_inp1 = ENABLE
        u.trigger = (Trigger.SRC_TENSOR_DONE, Trigger.NONE, Trigger.NONE)
        u.next_uop = (0, 0, 0)
        return u

    class _HandOp:
        name = "PFXDOT_ANT"
        subdim = False
        perf_max = 1

        def __init__(self):
            self.spec = Spec(body=scan(SAluOp.ADD, Src0 * Src1),
                             reference=_pfx_ref)
            self._compiled = {}

        def compile(self, ver):
            if ver in self._compiled:
                return self._compiled[ver]
            r = DveOpSpec(
                name=self.name,
                opcode=dom.get_dve_sub_opcode(self.name),
                uops=lower(self.spec, ver=ver),
                rd1_en=_has_src1(self.spec),
                uops_2x=[_seed_zero(), _steady_2x()],
                perf_max=1,
            )
            r.validate(ver)
            self._compiled[ver] = r
            return r

    op = _HandOp()
    dom.OPS.append(op)
    row = dom._CUSTOM_DVE_ROW_BASE + len(dom.OPS) - 1
    assert row < 0x20
    dom._SUB_OPCODE_FOR_NAME[op.name] = row
    dom.CUSTOM_DVE_SPECS[op.name] = op.spec
    dom._PFXDOT_ANT_OP = op
    return op


def _emit_pfxdot(nc, op, out, in0, in1):
    bi = nc.vector._custom_dve(op, out=out, in0=in0, in1=in1, s0=0.0)
    bi.ins.perf_max = op.perf_max
    return bi


def _patch_act_tables():
    """Make every activation func this kernel uses resolve to one table set
    (ACT_SET), so bacc emits a single ACT_TABLE_LOAD."""
    import concourse.bacc as bacc_mod
    import concourse.hw_specs as hw_specs
    from concourse import mybir

    if getattr(bacc_mod, "_blockattn_act_patch", False):
        return
    AF = mybir.ActivationFunctionType
    mine = {AF.Square, AF.Exp, AF.Ln, AF.Copy, AF.Identity}
    orig = hw_specs.get_activation_tables

    def patched(arch):
        t = dict(orig(arch))
        assert ACT_SET in t and mine <= t[ACT_SET], (ACT_SET, t.get(ACT_SET))
        return {
            name: (funcs if name == ACT_SET else funcs - mine)
            for name, funcs in t.items()
        }

    bacc_mod.get_activation_tables = patched
    bacc_mod._blockattn_act_patch = True


def _groups(q):
    """[(g, t0, tg)] active groups of oct q (t0 = core-local token base)."""
    out = []
    for g in range(QG):
        t0 = q * QTOK + g * GROUP
        tg = min(GROUP, TOK - t0)
        if tg > 0:
            out.append((g, t0, tg))
    return out


def build_nc():
    import concourse.bacc as bacc
    import concourse.tile as tile
    from concourse import mybir

    _patch_act_tables()
    pfx_op = _register_pfxdot()

    f32 = mybir.dt.float32
    bf16 = mybir.dt.bfloat16
    fp16 = mybir.dt.float16
    AF = mybir.ActivationFunctionType
    OP = mybir.AluOpType

    nc = bacc.Bacc("TRN2", target_bir_lowering=False, debug=False)

    vst_d = nc.dram_tensor("vstack", [NQUAD, ROWS, QG * D], bf16,
                           kind="ExternalInput")
    w2b_d = nc.dram_tensor("w2b", [ROWS, D], bf16, kind="ExternalInput")
    oh_d = nc.dram_tensor("onehot", [ROWS, GROUP], f32, kind="ExternalInput")
    ohT_d = nc.dram_tensor("onehotT", [GROUP, ROWS], f32, kind="ExternalInput")
    oh8_d = nc.dram_tensor("onehot8", [ROWS, QG * GROUP], bf16,
                           kind="ExternalInput")
    h_d = nc.dram_tensor("h", [TOK, D], bf16, kind="ExternalOutput")

    vst = vst_d.ap()
    hout = h_d.ap()

    with tile.TileContext(nc) as tc:
        import contextlib
        ctx = contextlib.ExitStack()
        with ctx:
            consts = ctx.enter_context(tc.tile_pool(name="consts", bufs=1))
            vq_pool = ctx.enter_context(tc.tile_pool(name="vq", bufs=5))
            dpfx_pool = ctx.enter_context(tc.tile_pool(name="dpfx", bufs=2))
            spfx_pool = ctx.enter_context(tc.tile_pool(name="spfx", bufs=2))
            stats_pool = ctx.enter_context(tc.tile_pool(name="stats", bufs=4))
            small_pool = ctx.enter_context(tc.tile_pool(name="small", bufs=4))
            hsb_pool = ctx.enter_context(tc.tile_pool(name="hsb", bufs=6))
            hpage_pool = ctx.enter_context(
                tc.tile_pool(name="hpage", bufs=3, space="PSUM"))
            zp_pool = ctx.enter_context(
                tc.tile_pool(name="zp", bufs=1, space="PSUM"))
            rzb_pool = ctx.enter_context(
                tc.tile_pool(name="rzb", bufs=1, space="PSUM"))

            w2b = consts.tile([ROWS, D], bf16)
            nc.sync.dma_start(w2b[:], w2b_d.ap()[:])
            oh = consts.tile([ROWS, GROUP], f32)
            nc.sync.dma_start(oh[:], oh_d.ap()[:])
            ohT = consts.tile([GROUP, ROWS], f32)
            nc.sync.dma_start(ohT[:], ohT_d.ap()[:])
            oh8 = consts.tile([ROWS, QG * GROUP], bf16)
            nc.sync.dma_start(oh8[:], oh8_d.ap()[:])
            # memsets on the (idle) Q7 engine: on the DVE queue they would
            # sit behind the first stat ops, and the FIRST ACT Square (plus
            # the ACT_TABLE_LOAD in front of it) waits on zero_col — which
            # kept ACT idle for ~9us at startup
            zero_col = consts.tile([ROWS, 1], f32)
            nc.gpsimd.memset(zero_col[:], 0.0)
            eps_col = consts.tile([ROWS, 1], f32)
            nc.gpsimd.memset(eps_col[:], EPS)
            # discard target for the ACT Square outputs (write-only; the
            # stat rides accum_out) — shared across octs, ACT serializes
            sq_discard = consts.tile([ROWS, D], bf16)

            def emit_load(q):
                """Tiles + input DMA for oct q (first oct chunked across the
                Sync and Q7 queues so stats can start early)."""
                groups = _groups(q)
                vq = vq_pool.tile([ROWS, QG * D], bf16)
                stats = stats_pool.tile([ROWS, QG], f32)
                dpfx = dpfx_pool.tile([ROWS, QG * D], fp16)
                spfx = spfx_pool.tile([ROWS, (QG - 2) * D], fp16)
                # two triggers per oct = two DMA queue slots moving in
                # parallel (one starves the stream); first oct split across
                # both trigger engines so compute starts asap
                if q == 0:
                    # small leading chunks on both queues so the first stat
                    # ops start as soon as one group lands; larger behind
                    plan = ((0, D, nc.sync), (D, D, nc.gpsimd),
                            (2 * D, 2 * D, nc.sync), (4 * D, 2 * D, nc.gpsimd),
                            (6 * D, 2 * D, nc.sync))
                    for c0, cw, eng in plan:
                        eng.dma_start(vq[:, c0:c0 + cw],
                                      vst[q][:, c0:c0 + cw])
                elif q < NQUAD - 1:
                    cw = len(groups) * D // 2
                    for ci in range(2):
                        nc.sync.dma_start(vq[:, ci * cw:(ci + 1) * cw],
                                          vst[q][:, ci * cw:(ci + 1) * cw])
                else:
                    used = len(groups) * D
                    nc.sync.dma_start(vq[:, 0:used], vst[q][:, 0:used])
                return vq, stats, dpfx, spfx

            def emit_passes(q, vq, stats, dpfx, spfx, part1=None, part2=None):
                """The stat passes for oct q; part1/part2 are emission hooks
                for the previous oct's chain + copy stages (interleaved so
                every engine's static order always has ready work)."""
                groups = _groups(q)
                k_act = QG if q == NQUAD - 1 else (2 if q == 0 else K_ACT)
                for i, (g, t0, tg) in enumerate(groups):
                    gc = g * D
                    # dot: custom 2x prefix op on DVE
                    _emit_pfxdot(nc, pfx_op, out=dpfx[:, gc:gc + D],
                                 in0=vq[0:ROWS, gc:gc + D], in1=w2b[0:ROWS, :])
                    # ssq: ACT Square+accum for the first k_act groups,
                    # else 2x prefix (in1 = in0) on DVE
                    if i < k_act:
                        nc.scalar.activation(
                            sq_discard[:], vq[0:ROWS, gc:gc + D], AF.Square,
                            bias=zero_col[:], accum_out=stats[:, g:g + 1])
                    else:
                        sc = (i - k_act) * D
                        _emit_pfxdot(nc, pfx_op, out=spfx[:, sc:sc + D],
                                     in0=vq[0:ROWS, gc:gc + D],
                                     in1=vq[0:ROWS, gc:gc + D])
                    if i == 1 and part1 is not None:
                        part1()
                        part1 = None
                    if i == 5 and part2 is not None:
                        part2()
                        part2 = None
                if part1 is not None:
                    part1()
                if part2 is not None:
                    part2()

            def emit_chain(q, vq, stats, dpfx, spfx):
                """Softmax smalls + PE weighted sum (part 1 of the chain)."""
                groups = _groups(q)
                ng = len(groups)
                k_act = QG if q == NQUAD - 1 else (2 if q == 0 else K_ACT)
                n_dve = max(0, ng - k_act)
                if n_dve > 0:
                    # ssq totals computed on DVE live at the prefix tails;
                    # gather them into the f32 stats tile (one tiny copy)
                    nc.vector.tensor_copy(
                        stats[:, k_act:k_act + n_dve],
                        spfx[:, D - 1:n_dve * D:D])
                lnq = small_pool.tile([ROWS, QG], f32, tag="lnq")
                nc.scalar.activation(lnq[:], stats[:, 0:QG], AF.Ln,
                                     bias=eps_col[:], scale=1.0 / D)
                rq = small_pool.tile([ROWS, QG], f32, tag="rq")
                nc.scalar.activation(rq[:], lnq[:], AF.Exp,
                                     bias=zero_col[:], scale=-0.5)
                lg = small_pool.tile([ROWS, QG], f32, tag="lg")
                nc.vector.tensor_tensor(
                    out=lg[:], in0=dpfx[:, D - 1:QG * D:D],
                    in1=rq[:], op=OP.mult)
                e_sb = small_pool.tile([ROWS, QG], f32, tag="e_sb")
                nc.scalar.activation(e_sb[:], lg[:], AF.Exp, bias=zero_col[:])

                zp = zp_pool.tile([GROUP, QG], f32)
                nc.tensor.matmul(zp[:], lhsT=oh[:], rhs=e_sb[:],
                                 start=True, stop=True)
                rz = small_pool.tile([GROUP, QG], f32, tag="rz")
                nc.vector.reciprocal(rz[:], zp[:])
                rzb = rzb_pool.tile([ROWS, QG], f32)
                nc.tensor.matmul(rzb[:], lhsT=ohT[:], rhs=rz[:],
                                 start=True, stop=True)
                wcol = small_pool.tile([ROWS, QG], bf16, tag="wcol")
                nc.vector.tensor_mul(wcol[:], e_sb[:], rzb[:])

                # ---- weighted sum via PE (bf16), 4 groups per PSUM page ----
                lhsTs = small_pool.tile([ROWS, QG * GROUP], bf16, tag="lhsTs")
                active_pages = sorted({g // 4 for g, _, _ in groups})
                hpages = {pg: hpage_pool.tile([128, D], f32, tag="hpage",
                                              name="hpage")
                          for pg in active_pages}
                nc.vector.tensor_tensor(
                    out=lhsTs[:, :].rearrange("p (g j) -> p g j", g=QG),
                    in0=oh8[:, :].rearrange("p (g j) -> p g j", g=QG),
                    in1=wcol[:, :].unsqueeze(2).to_broadcast(
                        [ROWS, QG, GROUP]),
                    op=OP.mult)
                for g, t0, tg in groups:
                    gc = g * D
                    lw = lhsTs[:, g * GROUP:(g + 1) * GROUP]
                    pg = g // 4
                    col = 32 * (g % 4)
                    for hh in range(2):
                        nc.tensor.matmul(
                            hpages[pg][col:col + GROUP,
                                       512 * hh:512 * hh + 512],
                            lhsT=lw,
                            rhs=vq[0:ROWS, gc + 512 * hh:gc + 512 * hh + 512],
                            start=True, stop=True,
                            tile_position=(0, col))

                return hpages, active_pages

            def emit_copies(q, vq, hpages, active_pages):
                """PSUM -> SBUF (f32 -> bf16, split ACT/DVE) -> HBM."""
                groups = _groups(q)
                last = q >= NQUAD - 2
                act_cols = ACT_COPY_COLS if not last else D
                for pg in active_pages:
                    h_sb = hsb_pool.tile([128, D], bf16, tag="h_sb")
                    if act_cols > 0:
                        nc.scalar.copy(h_sb[:, 0:act_cols],
                                       hpages[pg][:, 0:act_cols])
                    if act_cols < D:
                        nc.vector.tensor_copy(h_sb[:, act_cols:D],
                                              hpages[pg][:, act_cols:D])
                    # tail octs go via the Sync HWDGE queue: Q7's fixed
                    # ~7us dge_drain starts after gpsimd's LAST instruction,
                    # so tail triggers on Q7 would push the drain (and the
                    # kernel end) out by their full duration.  On Sync they
                    # overlap the drain instead.
                    dma_eng = nc.sync if last else nc.gpsimd
                    for g, t0, tg in groups:
                        if g // 4 != pg:
                            continue
                        dma_eng.dma_start(
                            hout[t0:t0 + tg, :],
                            h_sb[32 * (g % 4):32 * (g % 4) + tg, :])

            # software pipeline: input DMAs lead by LOOKAHEAD octs; oct q-1's
            # chain and copies are emitted inside oct q's stat-pass stream.
            LOOKAHEAD = 2
            loads = {}
            for q in range(min(LOOKAHEAD + 1, NQUAD)):
                loads[q] = emit_load(q)
            prev = None
            for q in range(NQUAD):
                if prev is None:
                    emit_passes(q, *loads[q])
                else:
                    pq = prev
                    box = {}

                    def part1(pq=pq, box=box):
                        box["pages"] = emit_chain(pq, *loads[pq])

                    def part2(pq=pq, box=box):
                        emit_copies(pq, loads[pq][0], *box["pages"])

                    emit_passes(q, *loads[q], part1=part1, part2=part2)
                if q + LOOKAHEAD + 1 < NQUAD:
                    loads[q + LOOKAHEAD + 1] = emit_load(q + LOOKAHEAD + 1)
                prev = q
            pages = emit_chain(prev, *loads[prev])
            emit_copies(prev, loads[prev][0], *pages)

    nc.compile()
    return nc


def _host_inputs(blocks, partial_block, proj_w, norm_w):
    """Slice + interleave per-core inputs (host-side, numpy only)."""
    import ml_dtypes
    bf = ml_dtypes.bfloat16
    blocks = np.ascontiguousarray(blocks, dtype=np.float32).reshape(N, B * T, D)
    partial = np.ascontiguousarray(partial_block, dtype=np.float32).reshape(B * T, D)
    w2 = (np.asarray(proj_w, np.float32) * np.asarray(norm_w, np.float32))
    w2b = np.ascontiguousarray(
        np.broadcast_to(w2.astype(bf), (ROWS, D)))
    oh = np.zeros((ROWS, GROUP), np.float32)
    for p in range(ROWS):
        oh[p, p % GROUP] = 1.0
    ohT = np.ascontiguousarray(oh.T)
    oh8 = np.ascontiguousarray(np.tile(oh, (1, QG)).astype(bf))

    pad_tok = NQUAD * QTOK
    in_maps = []
    for c in range(N_CORES):
        s = slice(c * TOK, (c + 1) * TOK)
        av = np.zeros((NB, pad_tok, D), bf)
        av[:N, :TOK] = blocks[:, s, :].astype(bf)
        av[N, :TOK] = partial[s, :].astype(bf)
        # vstack[q, 14n+t', g*D+d] = av[n, q*112 + g*14 + t', d]
        vst = av.reshape(NB, NQUAD, QG, GROUP, D)
        vst = np.ascontiguousarray(vst.transpose(1, 0, 3, 2, 4))
        vst = vst.reshape(NQUAD, ROWS, QG * D)
        in_maps.append({
            "vstack": vst,
            "w2b": w2b,
            "onehot": oh,
            "onehotT": ohT,
            "onehot8": oh8,
        })
    return in_maps


def kernel(blocks, partial_block, proj_w, norm_w):
    from concourse.bass_utils import run_bass_kernel_spmd

    if "nc" not in _CACHE:
        _CACHE["nc"] = build_nc()
    nc = _CACHE["nc"]
    in_maps = _host_inputs(blocks, partial_block, proj_w, norm_w)
    res = run_bass_kernel_spmd(nc, in_maps, core_ids=list(range(N_CORES)))
    h = np.concatenate(
        [np.asarray(res.results[c]["h"]).astype(np.float32)
         for c in range(N_CORES)], axis=0)
    return h.reshape(B, T, D)
